# revision 5
# baseline (speedup 1.0000x reference)
"""Path-signature kernel for Trainium2 (8 NeuronCores, batch-data-parallel).

Computation per batch element b (window W=64, time-augmented dim d=32):
  path  = [linspace(0,1,64) | features[b, t-63:t+1, :]]          (64, 32)
  lvl1  = path[-1] - path[0]                                     (32,)
  inc   = diff(path, axis=0)   prev = path[:-1]                  (63, 32)
  sig2  = inc^T @ prev                                           (32, 32)
  sig3  = einsum('ti,tj,tk->ijk', inc, prev, prev) / 63          (32, 32, 32)
  out   = concat(lvl1, sig2.ravel(), sig3.ravel())               (33824,)

Device mapping (per core, 256 batches, 2 batches per 128-partition tile,
partition r = bl*64 + t):
  - sig3 is symmetric in (j,k): triangle j-blocks of 4 rows (k >= 4*floor(j/4))
    -> 576 cols/tile instead of 1024.  Host mirrors the lower triangle.
  - The j=0 row (time column x prev_k) is a per-partition tensor_scalar
    (4x DVE mode); the rest is fp16 tensor_tensor in 2x mode with HP tiles
    interleaved in the last AP dim.
  - Blocks are laid out so the first 512 pp cols depend only on the early
    DVE ops; matmuls split [0:512] / [512:576] / sig2 on PSUM bank edges.
  - lhsT is compact (128, nt, 32): batch A on partitions 0:63, B on 64:127.
    Each (tile, col-range) issues two concurrent 64x32 matmuls via
    tile_position (row groups 0/64, col groups per output slice) - no
    zero-padded block-diagonal, half the lhsT DMA traffic.
  - ScalarE evacuates PSUM->SBUF per 2 pairs (fp32->fp16); output DMA rides
    the ACT ring, input DMA the SP ring.
  - lvl1, sig2 sqrt(63) scale, and sig3 mirror/unpermute are host-side.
"""

import numpy as np

import concourse.bass as bass
import concourse.mybir as mybir
import concourse.tile as tile
from concourse import bacc
from concourse.bass_utils import run_bass_kernel_spmd

F32 = mybir.dt.float32
F16 = mybir.dt.float16

N_CORES = 8
B_TOTAL = 2048
T_TOTAL = 1024
F_IN = 31
W = 64
D = 32
B_CORE = B_TOTAL // N_CORES      # 256
N_TILES = B_CORE // 2            # 128  (2 batches per tile)
N_PAIRS = N_TILES // 2           # 64   (4 batches per pair)
OUT_D = D + D * D + D ** 3       # 33824

BS = 4                           # j-block height in the triangle
TS0 = True                       # j=0 row via tensor_scalar (time col)
GRP = 8                          # pairs per DVE-packed group (HP = 2*grp)


def make_blocks(bs=BS, ts0=TS0):
    """Triangle blocks (kind, j0, k0, nj, w, off) in pp-column order.

    Column order puts a prefix summing to exactly 512 first so the [0:512]
    matmul never depends on the last DVE ops of a group.
    """
    blocks = []
    for r in range(D // bs):
        j0 = r * bs
        w = D - j0
        if r == 0 and ts0:
            blocks.append(["ts", 0, 0, 1, D, 0])
            if bs > 1:
                blocks.append(["tt", 1, 0, bs - 1, D, 0])
        else:
            blocks.append(["tt", j0, j0, bs, w, 0])
    # reorder so a prefix sums to 512 (greedy: keep order, defer blocks that
    # overshoot)
    first, rest, acc = [], [], 0
    for b in blocks:
        sz = b[3] * b[4]
        if acc + sz <= 512:
            first.append(b)
            acc += sz
        else:
            rest.append(b)
    assert acc == 512, f"no 512 prefix: {acc}"
    blocks = first + rest
    off = 0
    for b in blocks:
        b[5] = off
        off += b[3] * b[4]
    return [tuple(b) for b in blocks], off


BLOCKS, C_SIG3 = make_blocks()            # 576
C_TILE = C_SIG3 + D                       # 608 (+32 sig2 cols)
PS_PAIR = 1024                            # psum cols per pair (bank pad)


def build_program(n_pairs=N_PAIRS, repeat=1, loop=0, chunk=8, variant="full",
                  pp_bufs=4, ps_bufs=2, s3_bufs=4,
                  out_eng='scalar', n_islice=8, grp=None, bs=BS, ts0=TS0,
                  evac1=False, pk_first=False):
    """Build the single-core Bass program (SPMD across cores)."""
    if grp is None:
        grp = GRP
    HP = 2 * grp
    n_tiles = 2 * n_pairs
    ngrp = n_pairs // grp
    blocks, c_sig3 = make_blocks(bs, ts0)
    c_tile = c_sig3 + D
    nc = bacc.Bacc(None, target_bir_lowering=False)

    lhsT16_d = nc.dram_tensor("lhsT16", [128, n_tiles * 32], F16,
                              kind="ExternalInput")
    pk_d = nc.dram_tensor("pk", [128, ngrp * D * HP], F16,
                          kind="ExternalInput")
    tvec_d = nc.dram_tensor("tvec", [128, 1], F32, kind="ExternalInput")
    out3_d = nc.dram_tensor("out3", [128, n_pairs * c_tile], F16,
                            kind="ExternalOutput")

    with tile.TileContext(nc) as tc:
        with (
            tc.tile_pool(name="const", bufs=1) as const_pool,
            tc.tile_pool(name="pp", bufs=pp_bufs) as pp_pool,
            tc.tile_pool(name="s3", bufs=s3_bufs) as s3_pool,
            tc.tile_pool(name="ps3", bufs=ps_bufs, space=bass.MemorySpace.PSUM) as ps3_pool,
        ):
            lhsT16_all = const_pool.tile([128, n_tiles, 32], F16)
            pk_all = const_pool.tile([128, ngrp, D, HP], F16)
            tvec_all = const_pool.tile([128, 1], F32)

            CHUNK = chunk if n_pairs % chunk == 0 else n_pairs
            n_chunks = n_pairs // CHUNK
            assert CHUNK % grp == 0

            isl_l = min(n_islice, n_tiles)
            isl_p = min(n_islice, ngrp)

            def body():
                nc.sync.dma_start(tvec_all[:, :], tvec_d[:, :])
                for d in range(isl_l):
                    q = n_tiles // isl_l
                    tsl = slice(d * q, (d + 1) * q)
                    def dma_lhs():
                        nc.sync.dma_start(
                            lhsT16_all[:, tsl, :],
                            lhsT16_d[:, d * q * 32:(d + 1) * q * 32]
                            .rearrange("p (t m) -> p t m", m=32))
                    def dma_pk():
                        if d >= isl_p:
                            return
                        qg = ngrp // isl_p
                        gsl = slice(d * qg, (d + 1) * qg)
                        nc.sync.dma_start(
                            pk_all[:, gsl, :, :],
                            pk_d[:, d * qg * D * HP:(d + 1) * qg * D * HP]
                            .rearrange("p (t m h) -> p t m h", m=D, h=HP))
                    if pk_first:
                        dma_pk(); dma_lhs()
                    else:
                        dma_lhs(); dma_pk()

                for ch in range(n_chunks):
                    s3_buf = (None if variant in ("noevac", "dveonly") else
                              s3_pool.tile([128, CHUNK, c_tile], F16, tag="s3buf"))
                    for gl in range(CHUNK // grp):
                        g = ch * (CHUNK // grp) + gl
                        pp = pp_pool.tile([128, c_sig3, HP], F16, tag="pp")
                        for (kind, j0, k0, nj, w, off) in blocks:
                            if variant == "nodve":
                                continue
                            if kind == "ts":
                                nc.vector.tensor_scalar_mul(
                                    pp[:, off:off + w, :],
                                    pk_all[:, g, k0:k0 + w, :],
                                    tvec_all[:, 0:1])
                            else:
                                out = (pp[:, off:off + nj * w, :]
                                       .rearrange("p (j k) h -> p j k h", k=w))
                                in0 = (pk_all[:, g, j0:j0 + nj, :]
                                       .unsqueeze(2)
                                       .broadcast_to([128, nj, w, HP]))
                                in1 = (pk_all[:, g, k0:k0 + w, :]
                                       .unsqueeze(1)
                                       .broadcast_to([128, nj, w, HP]))
                                nc.vector.tensor_mul(out, in0, in1)
                        for s in range(grp if evac1 else grp // 2):
                            npair_t = 1 if evac1 else 2
                            ps3 = (None if variant == "dveonly" else
                                   ps3_pool.tile([128, npair_t, PS_PAIR], F32,
                                                 tag="ps3"))
                            if variant not in ("nope", "dveonly"):
                                # PE queue is in-order: issue all prefix-
                                # dependent (and dep-free sig2) matmuls
                                # before any [512:] one, so a late DVE op
                                # can't stall them.
                                for q in range(npair_t):
                                    for half in range(2):
                                        pig = npair_t * s + q
                                        h = pig * 2 + half
                                        t = 2 * (g * grp + pig) + half
                                        lo = 64 * half
                                        for bl in range(2):
                                            rows = slice(64 * bl, 64 * bl + 64)
                                            op = slice(lo + 32 * bl,
                                                       lo + 32 * bl + 32)
                                            tp = (64 * bl, lo + 32 * bl)
                                            nc.tensor.matmul(
                                                ps3[op, q, 0:512],
                                                lhsT16_all[rows, t, :],
                                                pp[rows, 0:512, h],
                                                tile_position=tp)
                                            nc.tensor.matmul(
                                                ps3[op, q, c_sig3:c_tile],
                                                lhsT16_all[rows, t, :],
                                                pk_all[rows, g, 0:D, h],
                                                tile_position=tp)
                                for q in range(npair_t):
                                    for half in range(2):
                                        pig = npair_t * s + q
                                        h = pig * 2 + half
                                        t = 2 * (g * grp + pig) + half
                                        lo = 64 * half
                                        for bl in range(2):
                                            rows = slice(64 * bl, 64 * bl + 64)
                                            op = slice(lo + 32 * bl,
                                                       lo + 32 * bl + 32)
                                            tp = (64 * bl, lo + 32 * bl)
                                            nc.tensor.matmul(
                                                ps3[op, q, 512:c_sig3],
                                                lhsT16_all[rows, t, :],
                                                pp[rows, 512:c_sig3, h],
                                                tile_position=tp)
                            if variant not in ("noevac", "dveonly"):
                                cbase = gl * grp + npair_t * s
                                nc.scalar.copy(
                                    s3_buf[:, cbase:cbase + npair_t, :],
                                    ps3[:, :, 0:c_tile])

                    if variant not in ("noevac", "nodma3", "dveonly"):
                        cw = CHUNK * c_tile
                        getattr(nc, out_eng).dma_start(
                            out3_d[:, ch * cw:(ch + 1) * cw], s3_buf[:])

            if loop:
                with tc.For_i(0, loop, 1):
                    body()
            else:
                for _rep in range(repeat):
                    body()

    nc.compile()
    return nc


def make_inputs_for_core(inc, prev_s, base, n_tiles, grp=None):
    """Pack host arrays into the partition-major device layouts.

    inc: (B, 64, 32) with zero row at t=63; prev_s = prev/sqrt(63) likewise.
    """
    if grp is None:
        grp = GRP
    nt = n_tiles
    HP = 2 * grp
    ngrp = nt // HP
    pk = np.zeros((128, ngrp, D, HP), dtype=np.float16)

    sl = slice(base, base + 2 * nt)
    # (nt, 2, 64, 32) -> per bl: (64, nt, 32)
    A = inc[sl].reshape(nt, 2, 64, 32).transpose(1, 2, 0, 3)
    S = prev_s[sl].reshape(nt, 2, 64, 32).transpose(1, 2, 0, 3)
    lhsT = np.concatenate([A[0], A[1]], axis=0)          # (128, nt, 32)
    for bl in range(2):
        rows = slice(64 * bl, 64 * bl + 64)
        # (64, nt, 32) -> (64, ngrp, h, 32) -> (64, ngrp, 32, h)
        S16 = (S[bl].astype(np.float16)
               .reshape(64, ngrp, HP, 32).transpose(0, 1, 3, 2))
        pk[rows] = S16
    c0 = np.float64(1.0) / (63.0 * np.sqrt(np.float64(63.0)))
    tv = np.zeros((128, 1), dtype=np.float32)
    t_idx = np.arange(63, dtype=np.float64)
    tv[0:63, 0] = (t_idx * c0).astype(np.float32)
    tv[64:127, 0] = tv[0:63, 0]
    return {
        "lhsT16": np.ascontiguousarray(lhsT).reshape(128, nt * 32).astype(np.float16),
        "pk": pk.reshape(128, ngrp * D * HP),
        "tvec": tv,
    }


def host_preprocess(features, t):
    t = int(t)
    start = max(0, t - W + 1)
    window = features[:, start:t + 1, :]
    cur = window.shape[1]
    if cur < W:
        pad = np.broadcast_to(window[:, 0:1, :], (window.shape[0], W - cur, F_IN))
        window = np.concatenate([pad, window], axis=1)
    B = window.shape[0]
    path = np.empty((B, W, D), dtype=np.float32)
    path[:, :, 0] = np.linspace(0.0, 1.0, W, dtype=np.float32)[None, :]
    path[:, :, 1:] = window

    inc = np.zeros((B, W, D), dtype=np.float32)
    inc[:, :W - 1] = path[:, 1:] - path[:, :-1]
    prev_s = np.zeros((B, W, D), dtype=np.float32)
    prev_s[:, :W - 1] = path[:, :W - 1] * np.float32(1.0 / np.sqrt(np.float32(W - 1)))
    lvl1 = path[:, -1, :] - path[:, 0, :]
    return inc, prev_s, lvl1


_PROGRAM = None

_TRIL = np.tril_indices(D, k=-1)


def unpack_core(o3):
    """Device out3 (128, n_pairs*C_TILE) fp16 -> (B_CORE, D*D + D^3) f32."""
    npair = o3.shape[1] // C_TILE
    v = o3.astype(np.float32).reshape(2, 2, D, npair, C_TILE)  # (h, bl, i, p, c)
    v = np.ascontiguousarray(v.transpose(3, 0, 1, 2, 4)).reshape(
        npair * 4, D, C_TILE)                                   # batch-major
    B = npair * 4
    sig3 = np.empty((B, D, D, D), dtype=np.float32)
    for (kind, j0, k0, nj, w, off) in BLOCKS:
        blk = v[:, :, off:off + nj * w].reshape(B, D, nj, w)
        sig3[:, :, j0:j0 + nj, k0:D] = blk
    sig2 = v[:, :, C_SIG3:C_TILE] * np.float32(np.sqrt(np.float64(63.0)))
    sig3[:, :, _TRIL[0], _TRIL[1]] = sig3[:, :, _TRIL[1], _TRIL[0]]
    return sig2.reshape(B, D * D), sig3.reshape(B, D ** 3)


def run(features, t, trace=False):
    global _PROGRAM
    features = np.asarray(features, dtype=np.float32)
    inc, prev_s, lvl1 = host_preprocess(features, t)

    if _PROGRAM is None:
        _PROGRAM = build_program()
    nc = _PROGRAM

    in_maps = [
        make_inputs_for_core(inc, prev_s, c * B_CORE, N_TILES)
        for c in range(N_CORES)
    ]
    res = run_bass_kernel_spmd(nc, in_maps, list(range(N_CORES)), trace=trace)
    out = np.empty((B_TOTAL, OUT_D), dtype=np.float32)
    out[:, 0:D] = lvl1
    for c in range(N_CORES):
        rows = slice(c * B_CORE, (c + 1) * B_CORE)
        s2, s3 = unpack_core(res.results[c]["out3"])
        out[rows, D:D + D * D] = s2
        out[rows, D + D * D:] = s3
    return out, res


def kernel(features, t):
    return run(features, t)[0]
